# revision 23
# baseline (speedup 1.0000x reference)
"""Fused LoRA-Linear (per-token adapter routing) for 8 TRN2 NeuronCores.

Strategy (v2, fp8 DoubleRow base GEMM):
  - Shard tokens: 8192 -> 1024 per core. Replicate weight/adapters.
  - Base GEMM in fp8 e4m3 with DoubleRow perf mode (2 K-rows per PE
    cell pass): W pre-quantized on host at scale 64 (W*64 ~ N(0,1)
    fits e4m3), x shipped as bf16 in DoubleRow layout and cast to fp8
    on the DVE engine. PSUM accumulates 64*(x@W).
  - LoRA path needs better-than-fp8 x: the bf16 x feeds the adapter
    prologue a_allT = A_cat @ x^T (A stacked 8x16 rows). ams =
    a_allT * (smask*64) keeps the LoRA delta in the 64x domain, so
    one extra bf16 K-step per output tile lands it in the same PSUM
    accumulation group as the base matmuls.
  - Drain: DVE copies PSUM*(1/64) to SBUF, adds bf16 bias, DMA out.
    n=0 runs k-outer with the prologue fused; n>=1 runs m-outer so
    each tile's drain hides under the next tile's matmuls.
  - HWDGE costs ~625ns per DMA instruction -> batch DMAs (A in 4
    groups, bt/bias in one shot, W per n-slice with the first n split
    in quarters only to unblock the PE sooner).
  - Measured vs fp32 reference (same rounding as this pipeline,
    simulated in numpy): rel err 1.80e-2 < 2e-2 gate. Deterministic:
    fp8 bits are produced host-side/on-chip with RNE; PE multiplies
    e4m3 exactly and accumulates fp32.
"""

import numpy as np
import ml_dtypes

import concourse.bass as bass
import concourse.bacc as bacc
import concourse.mybir as mybir
import concourse.tile as tile
from concourse.bass_utils import run_bass_kernel_spmd

SEQ, D_IN, D_OUT, RANK, N_ADAPTERS = 8192, 4096, 4096, 16, 8
N_CORES = 8
T = SEQ // N_CORES          # 1024 tokens per core
P = 128                     # partitions
FD = 512                    # psum tile free dim
KO = D_IN // P              # 32 single contraction tiles (prologue)
KO2 = D_IN // (2 * P)       # 16 DoubleRow contraction tiles (base)
NT = D_OUT // FD            # 8 output column chunks
MT = T // P                 # 8 token tiles per core
J = N_ADAPTERS * RANK       # 128 stacked adapter rows
AG = 4                      # A-tensor DMA groups
KOG = KO // AG              # ko's per A group (8)
WSCALE = 64.0               # fp8 scale for W (W*64 ~ N(0,1))
F32 = mybir.dt.float32
BF16 = mybir.dt.bfloat16
FP8 = mybir.dt.float8e4
DR = mybir.MatmulPerfMode.DoubleRow

_NC_CACHE = {}


def _build_nc():
    if "nc" in _NC_CACHE:
        return _NC_CACHE["nc"]
    nc = bacc.Bacc(None, target_bir_lowering=False, debug=False)
    xbf = nc.dram_tensor("xbf", [KO2, P, 2, T], BF16, kind="ExternalInput")
    w = nc.dram_tensor("w", [NT, P, KO2, 2, FD], FP8, kind="ExternalInput")
    biasb = nc.dram_tensor("biasb", [P, NT, FD], BF16, kind="ExternalInput")
    at = nc.dram_tensor("at", [AG, P, KOG, J], BF16, kind="ExternalInput")
    bt = nc.dram_tensor("bt", [J, NT, FD], BF16, kind="ExternalInput")
    smask = nc.dram_tensor("smask", [J, T], F32, kind="ExternalInput")
    out = nc.dram_tensor("out", [T, D_OUT], F32, kind="ExternalOutput")

    with tile.TileContext(nc) as tc:
        with (
            tc.tile_pool(name="xqp", bufs=1) as xq_pool,
            tc.tile_pool(name="xbp", bufs=8) as xbf_pool,
            tc.tile_pool(name="wp", bufs=3) as w_pool,
            tc.tile_pool(name="ap", bufs=2) as a_pool,
            tc.tile_pool(name="outp", bufs=8) as out_pool,
            tc.tile_pool(name="misc", bufs=1) as misc_pool,
            tc.tile_pool(name="psum", bufs=8, space="PSUM") as psum_pool,
        ):
            xbf_v = xbf[:]
            w_v = w[:]
            at_v = at[:]
            out_v = out[:]

            # tiles only; DMAs issued mid-phase-A (needed first at k2=15)
            smask_sb = misc_pool.tile([J, T], F32, tag="smask")
            bt_sb = misc_pool.tile([J, NT, FD], BF16, tag="bt")
            bias_sb = misc_pool.tile([P, NT, FD], BF16, tag="bias")
            # a_allT * smask64 (bf16: matmul can't mix 32-bit and 16-bit ins)
            ams = misc_pool.tile([J, T], BF16, tag="ams")
            # resident fp8 x in DoubleRow layout [p, k2, j, t]
            xq = xq_pool.tile([P, KO2, 2, T], FP8, tag="xq")

            NCH = T // FD  # a_allT token chunks (2)
            psa = [None] * NCH

            # PE warmup: ~5us of throwaway matmuls during the initial DMA
            # wait so the p-state/HAM ramp is spent before real work
            warm_sb = misc_pool.tile([P, FD + P], BF16, tag="warm")
            nc.vector.memset(warm_sb[:], 0.0)
            ps_warm = psum_pool.tile([P, FD], F32, tag="ps", name="ps_warm")
            NWARM = 12  # ~2.6us, spans the first xbf chunk's DMA latency
            for wi in range(NWARM):
                nc.tensor.matmul(
                    ps_warm[:], warm_sb[:, FD:], warm_sb[:, :FD],
                    start=(wi == 0), stop=(wi == NWARM - 1),
                )

            def drain_tile(ps, m, n, chunks=1):
                o_sb = out_pool.tile([P, FD], F32, tag="o")
                cf = FD // chunks
                for ci in range(chunks):
                    sl = slice(ci * cf, (ci + 1) * cf)
                    # o = psum/64 + bias, single DVE op
                    nc.vector.scalar_tensor_tensor(
                        out=o_sb[:, sl], in0=ps[:, sl], scalar=1.0 / WSCALE,
                        in1=bias_sb[:, n, sl],
                        op0=mybir.AluOpType.mult, op1=mybir.AluOpType.add,
                    )
                    nc.sync.dma_start(
                        out_v[m * P:(m + 1) * P,
                              n * FD + ci * cf:n * FD + (ci + 1) * cf],
                        o_sb[:, sl],
                    )

            for n in range(NT):
                w_sb = w_pool.tile([P, KO2, 2, FD], FP8, tag="w")
                if n != 0:
                    nc.sync.dma_start(w_sb[:], w_v[n])
                if n == 1:
                    nc.sync.dma_start(bt_sb[:, 1:], bt[:, 1:])
                    nc.sync.dma_start(bias_sb[:, 1:], biasb[:, 1:])
                if n == 0:
                    # k-outer: prologue (A @ x^T, bf16) + x cast + base
                    # matmuls for m 0..5 share the k sweep; m 6,7 follow in
                    # a second sweep whose matmuls hide the m 0..5 drains.
                    for c in range(NCH):
                        psa[c] = psum_pool.tile([J, FD], F32, tag="ps",
                                                name=f"psa_{c}")
                    pss = {m: psum_pool.tile([P, FD], F32, tag="ps",
                                             name=f"ps_0_{m}")
                           for m in range(6)}
                    PF = 4  # xbf chunks prefetched ahead
                    xb_tiles = {}

                    def fetch_xb(k2):
                        xb_tiles[k2] = xbf_pool.tile(
                            [P, 2, T], BF16, tag="xb", name=f"xb_{k2}"
                        )
                        # j-halves land separately: prologue j=0 starts on
                        # half the chunk
                        nc.sync.dma_start(xb_tiles[k2][:, 0], xbf_v[k2, :, 0])
                        nc.sync.dma_start(xb_tiles[k2][:, 1], xbf_v[k2, :, 1])

                    def fetch_a(g):
                        t_ = a_pool.tile([P, KOG, J], BF16, tag="a",
                                         name=f"a_{g}")
                        nc.sync.dma_start(t_[:], at_v[g])
                        return t_

                    def fetch_wq(q):
                        nc.sync.dma_start(
                            w_sb[:, 4 * q:4 * (q + 1)],
                            w_v[n, :, 4 * q:4 * (q + 1)],
                        )

                    # critical-path-ordered start: x0, A0, x1, Wq0, x2, x3
                    fetch_xb(0)
                    a_tiles = {0: fetch_a(0)}
                    fetch_xb(1)
                    fetch_wq(0)
                    fetch_xb(2)
                    fetch_xb(3)
                    for k2 in range(KO2):
                        last_k = k2 == KO2 - 1
                        if k2 + PF < KO2:
                            fetch_xb(k2 + PF)
                        xb_sb = xb_tiles.pop(k2)
                        # cast bf16 -> fp8 on DVE (1.07us < 1.49us PE pace),
                        # per j-half so each starts as its DMA lands
                        nc.vector.tensor_copy(xq[:, k2, 0], xb_sb[:, 0])
                        nc.vector.tensor_copy(xq[:, k2, 1], xb_sb[:, 1])
                        for j in range(2):
                            ko = 2 * k2 + j
                            a_sb = a_tiles[ko // KOG]
                            for c in range(NCH):
                                nc.tensor.matmul(
                                    psa[c][:], a_sb[:, ko % KOG],
                                    xb_sb[:, j, c * FD:(c + 1) * FD],
                                    start=(ko == 0), stop=(ko == KO - 1),
                                )
                        # stagger non-critical DMAs behind the x stream:
                        # A groups and w quarters ahead of their consumers,
                        # the k2=15-consumed tensors (smask/bt/bias) late
                        if k2 in (2, 6, 10):
                            fetch_wq(k2 // 4 + 1)
                        if k2 in (3, 7, 11):
                            g = k2 // 4 + 1
                            a_tiles[g] = fetch_a(g)
                        elif k2 == 9:
                            nc.sync.dma_start(smask_sb[:], smask[:])
                        elif k2 == 12:
                            # phase A only needs the n=0 slices; the rest
                            # streams at the start of phase B
                            nc.sync.dma_start(bt_sb[:, 0], bt[:, 0])
                        elif k2 == 13:
                            nc.sync.dma_start(bias_sb[:, 0], biasb[:, 0])
                        if last_k:
                            for c in range(NCH):
                                nc.vector.tensor_mul(
                                    out=ams[:, c * FD:(c + 1) * FD],
                                    in0=psa[c][:],
                                    in1=smask_sb[:, c * FD:(c + 1) * FD],
                                )
                        for m in range(6):
                            nc.tensor.matmul(
                                pss[m][:], xq[:, k2, :, m * P:(m + 1) * P],
                                w_sb[:, k2], start=(k2 == 0), stop=False,
                                perf_mode=DR,
                            )
                            if last_k:
                                nc.tensor.matmul(
                                    pss[m][:], ams[:, m * P:(m + 1) * P],
                                    bt_sb[:, n], start=False, stop=True,
                                )
                                drain_tile(pss[m], m, n)
                    for m in range(6, MT):
                        ps = psum_pool.tile([P, FD], F32, tag="ps",
                                            name=f"ps_0_{m}")
                        for k2 in range(KO2):
                            nc.tensor.matmul(
                                ps[:], xq[:, k2, :, m * P:(m + 1) * P],
                                w_sb[:, k2], start=(k2 == 0), stop=False,
                                perf_mode=DR,
                            )
                        nc.tensor.matmul(
                            ps[:], ams[:, m * P:(m + 1) * P], bt_sb[:, n],
                            start=False, stop=True,
                        )
                        drain_tile(ps, m, n)
                else:
                    # m-outer: each tile closes right after its k sweep, so
                    # DVE drains + out DMA hide under the next tile's matmuls
                    for m in range(MT):
                        ps = psum_pool.tile([P, FD], F32, tag="ps",
                                            name=f"ps_{n}_{m}")
                        for k2 in range(KO2):
                            nc.tensor.matmul(
                                ps[:], xq[:, k2, :, m * P:(m + 1) * P],
                                w_sb[:, k2], start=(k2 == 0), stop=False,
                                perf_mode=DR,
                            )
                        nc.tensor.matmul(
                            ps[:], ams[:, m * P:(m + 1) * P], bt_sb[:, n],
                            start=False, stop=True,
                        )
                        last_tile = (n == NT - 1 and m == MT - 1)
                        drain_tile(ps, m, n, chunks=2 if last_tile else 1)

    nc.compile()
    _NC_CACHE["nc"] = nc
    return nc


def _prep_in_maps(x, weight, bias, A_buffer, B_buffer, scalings, token_indices):
    x = np.asarray(x, np.float32)
    weight = np.asarray(weight, np.float32)
    bias = np.asarray(bias, np.float32)
    A_buffer = np.asarray(A_buffer, np.float32)
    B_buffer = np.asarray(B_buffer, np.float32)
    scalings = np.asarray(scalings, np.float32)
    token_indices = np.asarray(token_indices)

    # x^T in DoubleRow layout [k2, p, j, t], bf16
    xT = x.T.reshape(KO2, 2, P, SEQ).transpose(0, 2, 1, 3)
    xbf_full = np.ascontiguousarray(xT.astype(ml_dtypes.bfloat16))
    # W*64 quantized to e4m3, DoubleRow layout [n, p, k2, j, f]
    wq = np.clip(weight * WSCALE, -240, 240).astype(ml_dtypes.float8_e4m3)
    w_t = np.ascontiguousarray(
        wq.reshape(KO2, 2, P, NT, FD).transpose(3, 2, 0, 1, 4)
    )
    biasb = np.ascontiguousarray(
        np.broadcast_to(
            bias.reshape(1, NT, FD), (P, NT, FD)
        ).astype(ml_dtypes.bfloat16)
    )
    A_cat = A_buffer.reshape(J, D_IN)
    # [AG, P, KOG, J]: per-group contiguous per partition
    at = np.ascontiguousarray(
        A_cat.T.reshape(AG, KOG, P, J).transpose(0, 2, 1, 3)
        .astype(ml_dtypes.bfloat16)
    )
    bt = np.ascontiguousarray(
        B_buffer.transpose(0, 2, 1).reshape(J, NT, FD).astype(ml_dtypes.bfloat16)
    )
    adapter_of_row = (np.arange(J) // RANK).astype(token_indices.dtype)
    smask_full = (
        (token_indices[None, :] == adapter_of_row[:, None]).astype(np.float32)
        * (scalings[None, :] * np.float32(WSCALE))
    )  # [J, SEQ], includes the 64x domain scale

    in_maps = []
    for c in range(N_CORES):
        sl = slice(c * T, (c + 1) * T)
        in_maps.append({
            "xbf": np.ascontiguousarray(xbf_full[:, :, :, sl]),
            "w": w_t,
            "biasb": biasb,
            "at": at,
            "bt": bt,
            "smask": np.ascontiguousarray(smask_full[:, sl]),
        })
    return in_maps


def _run(inputs, trace=False):
    nc = _build_nc()
    in_maps = _prep_in_maps(**inputs)
    res = run_bass_kernel_spmd(
        nc, in_maps, core_ids=list(range(N_CORES)), trace=trace
    )
    out = np.concatenate([r["out"] for r in res.results], axis=0)
    return out, res


def kernel(**inputs) -> np.ndarray:
    out, _ = _run(inputs, trace=False)
    return out


# revision 36
# speedup vs baseline: 1.0020x; 1.0020x over previous
"""Fused LoRA-Linear (per-token adapter routing) for 8 TRN2 NeuronCores.

Strategy (v2, fp8 DoubleRow base GEMM):
  - Shard tokens: 8192 -> 1024 per core. Replicate weight/adapters.
  - Base GEMM in fp8 e4m3 with DoubleRow perf mode (2 K-rows per PE
    cell pass): W pre-quantized on host at scale 64 (W*64 ~ N(0,1)
    fits e4m3), x shipped as bf16 in DoubleRow layout and cast to fp8
    on the DVE engine. PSUM accumulates 64*(x@W).
  - LoRA path needs better-than-fp8 x: the bf16 x feeds the adapter
    prologue a_allT = A_cat @ x^T (A stacked 8x16 rows). ams =
    a_allT * (smask*64) keeps the LoRA delta in the 64x domain, so
    one extra bf16 K-step per output tile lands it in the same PSUM
    accumulation group as the base matmuls.
  - Drain: DVE copies PSUM*(1/64) to SBUF, adds bf16 bias, DMA out.
    n=0 runs k-outer with the prologue fused; n>=1 runs m-outer so
    each tile's drain hides under the next tile's matmuls.
  - HWDGE costs ~625ns per DMA instruction -> batch DMAs (A in 4
    groups, W per n-slice with the first split in quarters only to
    unblock the PE sooner). Phase A (n=0 sweep) is DMA-feed-bound on
    the 8MB x stream, so its DMA issue order is critical-path-first
    (x chunk 0, A group 0, W quarter 0) with everything consumed at
    k2=15 (smask, bt/bias n=0 slices) staggered late and the bulk
    bt/bias + w[1] queued at the phase boundary. A 12-matmul warmup
    on zeroed SBUF absorbs the initial DMA latency + PE ramp.
  - Measured on hardware vs the fp32 reference: rel err 1.80e-2 <
    2e-2 gate. Deterministic: fp8 bits are produced host-side/on-chip
    with RNE; PE multiplies e4m3 exactly and accumulates fp32.
"""

import numpy as np
import ml_dtypes

import concourse.bass as bass
import concourse.bacc as bacc
import concourse.mybir as mybir
import concourse.tile as tile
from concourse.bass_utils import run_bass_kernel_spmd

SEQ, D_IN, D_OUT, RANK, N_ADAPTERS = 8192, 4096, 4096, 16, 8
N_CORES = 8
T = SEQ // N_CORES          # 1024 tokens per core
P = 128                     # partitions
FD = 512                    # psum tile free dim
KO = D_IN // P              # 32 single contraction tiles (prologue)
KO2 = D_IN // (2 * P)       # 16 DoubleRow contraction tiles (base)
NT = D_OUT // FD            # 8 output column chunks
MT = T // P                 # 8 token tiles per core
J = N_ADAPTERS * RANK       # 128 stacked adapter rows
AG = 4                      # A-tensor DMA groups
KOG = KO // AG              # ko's per A group (8)
WSCALE = 64.0               # fp8 scale for W (W*64 ~ N(0,1))
F32 = mybir.dt.float32
BF16 = mybir.dt.bfloat16
FP8 = mybir.dt.float8e4
DR = mybir.MatmulPerfMode.DoubleRow

_NC_CACHE = {}


def _build_nc():
    if "nc" in _NC_CACHE:
        return _NC_CACHE["nc"]
    nc = bacc.Bacc(None, target_bir_lowering=False, debug=False)
    xbf = nc.dram_tensor("xbf", [KO2, P, 2, T], BF16, kind="ExternalInput")
    w = nc.dram_tensor("w", [NT, P, KO2, 2, FD], FP8, kind="ExternalInput")
    biasb = nc.dram_tensor("biasb", [P, NT, FD], BF16, kind="ExternalInput")
    at = nc.dram_tensor("at", [AG, P, KOG, J], BF16, kind="ExternalInput")
    bt = nc.dram_tensor("bt", [J, NT, FD], BF16, kind="ExternalInput")
    smask = nc.dram_tensor("smask", [J, T], BF16, kind="ExternalInput")
    out = nc.dram_tensor("out", [T, D_OUT], F32, kind="ExternalOutput")

    with tile.TileContext(nc) as tc:
        with (
            tc.tile_pool(name="xqp", bufs=1) as xq_pool,
            tc.tile_pool(name="xbp", bufs=8) as xbf_pool,
            tc.tile_pool(name="wp", bufs=3) as w_pool,
            tc.tile_pool(name="ap", bufs=2) as a_pool,
            tc.tile_pool(name="outp", bufs=8) as out_pool,
            tc.tile_pool(name="misc", bufs=1) as misc_pool,
            tc.tile_pool(name="psum", bufs=8, space="PSUM") as psum_pool,
        ):
            xbf_v = xbf[:]
            w_v = w[:]
            at_v = at[:]
            out_v = out[:]

            # tiles only; DMAs issued mid-phase-A (needed first at k2=15)
            smask_sb = misc_pool.tile([J, T], BF16, tag="smask")
            bt_sb = misc_pool.tile([J, NT, FD], BF16, tag="bt")
            bias_sb = misc_pool.tile([P, NT, FD], BF16, tag="bias")
            # a_allT * smask64 (bf16: matmul can't mix 32-bit and 16-bit ins)
            ams = misc_pool.tile([J, T], BF16, tag="ams")
            # resident fp8 x in DoubleRow layout [p, k2, j, t]
            xq = xq_pool.tile([P, KO2, 2, T], FP8, tag="xq")

            NCH = T // FD  # a_allT token chunks (2)
            psa = [None] * NCH

            # PE warmup: ~5us of throwaway matmuls during the initial DMA
            # wait so the p-state/HAM ramp is spent before real work
            warm_sb = misc_pool.tile([P, FD + P], BF16, tag="warm")
            nc.vector.memset(warm_sb[:], 0.0)
            ps_warm = psum_pool.tile([P, FD], F32, tag="ps", name="ps_warm")
            NWARM = 6  # memset chain (~1.6us) + 6 mms ends as xbf[0] lands
            for wi in range(NWARM):
                nc.tensor.matmul(
                    ps_warm[:], warm_sb[:, FD:], warm_sb[:, :FD],
                    start=(wi == 0), stop=(wi == NWARM - 1),
                )

            def drain_tile(ps, m, n, chunks=1):
                o_sb = out_pool.tile([P, FD], F32, tag="o")
                cf = FD // chunks
                for ci in range(chunks):
                    sl = slice(ci * cf, (ci + 1) * cf)
                    # o = psum/64 + bias, single DVE op
                    nc.vector.scalar_tensor_tensor(
                        out=o_sb[:, sl], in0=ps[:, sl], scalar=1.0 / WSCALE,
                        in1=bias_sb[:, n, sl],
                        op0=mybir.AluOpType.mult, op1=mybir.AluOpType.add,
                    )
                    nc.sync.dma_start(
                        out_v[m * P:(m + 1) * P,
                              n * FD + ci * cf:n * FD + (ci + 1) * cf],
                        o_sb[:, sl],
                    )

            w1_sb = None  # fetched at the end of phase A
            for n in range(NT):
                if n == 1:
                    w_sb = w1_sb
                    nc.sync.dma_start(bt_sb[:, 1:], bt[:, 1:])
                    nc.sync.dma_start(bias_sb[:, 1:], biasb[:, 1:])
                else:
                    w_sb = w_pool.tile([P, KO2, 2, FD], FP8, tag="w",
                                       name=f"w_{n}")
                    if n != 0:
                        nc.sync.dma_start(w_sb[:], w_v[n])

                if n == 0:
                    # k-outer: prologue (A @ x^T, bf16) + x cast + base
                    # matmuls for m 0..5 share the k sweep; m 6,7 follow in
                    # a second sweep whose matmuls hide the m 0..5 drains.
                    # Base matmuls trail the prologue by LAG k2-steps so
                    # they never wait on a just-issued W quarter.
                    LAG = 2
                    for c in range(NCH):
                        psa[c] = psum_pool.tile([J, FD], F32, tag="ps",
                                                name=f"psa_{c}")
                    pss = {m: psum_pool.tile([P, FD], F32, tag="ps",
                                             name=f"ps_0_{m}")
                           for m in range(6)}
                    PF = 4  # xbf chunks prefetched ahead
                    xb_tiles = {}

                    def fetch_xb(k2):
                        xb_tiles[k2] = xbf_pool.tile(
                            [P, 2, T], BF16, tag="xb", name=f"xb_{k2}"
                        )
                        # j-halves land separately: prologue j=0 starts on
                        # half the chunk
                        nc.sync.dma_start(xb_tiles[k2][:, 0], xbf_v[k2, :, 0])
                        nc.sync.dma_start(xb_tiles[k2][:, 1], xbf_v[k2, :, 1])

                    def fetch_a(g):
                        t_ = a_pool.tile([P, KOG, J], BF16, tag="a",
                                         name=f"a_{g}")
                        nc.sync.dma_start(t_[:], at_v[g])
                        return t_

                    def fetch_wq(q):
                        nc.sync.dma_start(
                            w_sb[:, 4 * q:4 * (q + 1)],
                            w_v[n, :, 4 * q:4 * (q + 1)],
                        )

                    # critical-path-ordered start: x0, A0, x1, Wq0, x2, x3
                    fetch_xb(0)
                    a_tiles = {0: fetch_a(0)}
                    fetch_xb(1)
                    fetch_wq(0)
                    fetch_xb(2)
                    fetch_xb(3)
                    for k2 in range(KO2):
                        last_k = k2 == KO2 - 1
                        if k2 + PF < KO2:
                            fetch_xb(k2 + PF)
                        xb_sb = xb_tiles.pop(k2)
                        # cast bf16 -> fp8 on DVE (1.07us < 1.49us PE pace),
                        # per j-half so each starts as its DMA lands
                        nc.vector.tensor_copy(xq[:, k2, 0], xb_sb[:, 0])
                        nc.vector.tensor_copy(xq[:, k2, 1], xb_sb[:, 1])
                        for j in range(2):
                            ko = 2 * k2 + j
                            a_sb = a_tiles[ko // KOG]
                            for c in range(NCH):
                                nc.tensor.matmul(
                                    psa[c][:], a_sb[:, ko % KOG],
                                    xb_sb[:, j, c * FD:(c + 1) * FD],
                                    start=(ko == 0), stop=(ko == KO - 1),
                                )
                        # stagger non-critical DMAs behind the x stream:
                        # A groups and w quarters ahead of their consumers,
                        # the k2=15-consumed tensors (smask/bt/bias) late
                        if k2 in (2, 6, 10):
                            fetch_wq(k2 // 4 + 1)
                        if k2 in (3, 7, 11):
                            g = k2 // 4 + 1
                            a_tiles[g] = fetch_a(g)
                        elif k2 == 9:
                            nc.sync.dma_start(smask_sb[:], smask[:])
                        elif k2 == 12:
                            # phase A only needs the n=0 slices; the rest
                            # streams once A-critical bytes are all queued
                            nc.sync.dma_start(bt_sb[:, 0], bt[:, 0])
                        elif k2 == 13:
                            nc.sync.dma_start(bias_sb[:, 0], biasb[:, 0])
                        if last_k:
                            for c in range(NCH):
                                nc.vector.tensor_mul(
                                    out=ams[:, c * FD:(c + 1) * FD],
                                    in0=psa[c][:],
                                    in1=smask_sb[:, c * FD:(c + 1) * FD],
                                )
                        if k2 >= LAG:
                            k2b = k2 - LAG
                            for m in range(6):
                                nc.tensor.matmul(
                                    pss[m][:], xq[:, k2b, :, m * P:(m + 1) * P],
                                    w_sb[:, k2b], start=(k2b == 0), stop=False,
                                    perf_mode=DR,
                                )
                    for k2b in range(KO2 - LAG, KO2):
                        for m in range(6):
                            nc.tensor.matmul(
                                pss[m][:], xq[:, k2b, :, m * P:(m + 1) * P],
                                w_sb[:, k2b], start=(k2b == 0), stop=False,
                                perf_mode=DR,
                            )
                    # w[1] next in the DMA queue, ahead of the drain
                    # out-DMAs: lands during the g2 sweep, so n=1 never waits
                    w1_sb = w_pool.tile([P, KO2, 2, FD], FP8, tag="w",
                                        name="w_1")
                    nc.sync.dma_start(w1_sb[:], w_v[1])
                    for m in range(6):
                        nc.tensor.matmul(
                            pss[m][:], ams[:, m * P:(m + 1) * P],
                            bt_sb[:, n], start=False, stop=True,
                        )
                        drain_tile(pss[m], m, n)
                    for m in range(6, MT):
                        ps = psum_pool.tile([P, FD], F32, tag="ps",
                                            name=f"ps_0_{m}")
                        for k2 in range(KO2):
                            nc.tensor.matmul(
                                ps[:], xq[:, k2, :, m * P:(m + 1) * P],
                                w_sb[:, k2], start=(k2 == 0), stop=False,
                                perf_mode=DR,
                            )
                        nc.tensor.matmul(
                            ps[:], ams[:, m * P:(m + 1) * P], bt_sb[:, n],
                            start=False, stop=True,
                        )
                        drain_tile(ps, m, n)
                else:
                    # m-outer: each tile closes right after its k sweep, so
                    # DVE drains + out DMA hide under the next tile's matmuls
                    for m in range(MT):
                        ps = psum_pool.tile([P, FD], F32, tag="ps",
                                            name=f"ps_{n}_{m}")
                        for k2 in range(KO2):
                            nc.tensor.matmul(
                                ps[:], xq[:, k2, :, m * P:(m + 1) * P],
                                w_sb[:, k2], start=(k2 == 0), stop=False,
                                perf_mode=DR,
                            )
                        nc.tensor.matmul(
                            ps[:], ams[:, m * P:(m + 1) * P], bt_sb[:, n],
                            start=False, stop=True,
                        )
                        drain_tile(ps, m, n)

    nc.compile()
    _NC_CACHE["nc"] = nc
    return nc


def _prep_in_maps(x, weight, bias, A_buffer, B_buffer, scalings, token_indices):
    x = np.asarray(x, np.float32)
    weight = np.asarray(weight, np.float32)
    bias = np.asarray(bias, np.float32)
    A_buffer = np.asarray(A_buffer, np.float32)
    B_buffer = np.asarray(B_buffer, np.float32)
    scalings = np.asarray(scalings, np.float32)
    token_indices = np.asarray(token_indices)

    # x^T in DoubleRow layout [k2, p, j, t], bf16
    xT = x.T.reshape(KO2, 2, P, SEQ).transpose(0, 2, 1, 3)
    xbf_full = np.ascontiguousarray(xT.astype(ml_dtypes.bfloat16))
    # W*64 quantized to e4m3, DoubleRow layout [n, p, k2, j, f]
    wq = np.clip(weight * WSCALE, -240, 240).astype(ml_dtypes.float8_e4m3)
    w_t = np.ascontiguousarray(
        wq.reshape(KO2, 2, P, NT, FD).transpose(3, 2, 0, 1, 4)
    )
    biasb = np.ascontiguousarray(
        np.broadcast_to(
            bias.reshape(1, NT, FD), (P, NT, FD)
        ).astype(ml_dtypes.bfloat16)
    )
    A_cat = A_buffer.reshape(J, D_IN)
    # [AG, P, KOG, J]: per-group contiguous per partition
    at = np.ascontiguousarray(
        A_cat.T.reshape(AG, KOG, P, J).transpose(0, 2, 1, 3)
        .astype(ml_dtypes.bfloat16)
    )
    bt = np.ascontiguousarray(
        B_buffer.transpose(0, 2, 1).reshape(J, NT, FD).astype(ml_dtypes.bfloat16)
    )
    adapter_of_row = (np.arange(J) // RANK).astype(token_indices.dtype)
    smask_full = (
        (token_indices[None, :] == adapter_of_row[:, None]).astype(np.float32)
        * (scalings[None, :] * np.float32(WSCALE))
    ).astype(ml_dtypes.bfloat16)  # [J, SEQ], includes the 64x domain scale

    in_maps = []
    for c in range(N_CORES):
        sl = slice(c * T, (c + 1) * T)
        in_maps.append({
            "xbf": np.ascontiguousarray(xbf_full[:, :, :, sl]),
            "w": w_t,
            "biasb": biasb,
            "at": at,
            "bt": bt,
            "smask": np.ascontiguousarray(smask_full[:, sl]),
        })
    return in_maps


def _run(inputs, trace=False):
    nc = _build_nc()
    in_maps = _prep_in_maps(**inputs)
    res = run_bass_kernel_spmd(
        nc, in_maps, core_ids=list(range(N_CORES)), trace=trace
    )
    out = np.concatenate([r["out"] for r in res.results], axis=0)
    return out, res


def kernel(**inputs) -> np.ndarray:
    out, _ = _run(inputs, trace=False)
    return out


# revision 37
# speedup vs baseline: 1.0022x; 1.0002x over previous
"""Fused LoRA-Linear (per-token adapter routing) for 8 TRN2 NeuronCores.

Strategy (v2, fp8 DoubleRow base GEMM):
  - Shard tokens: 8192 -> 1024 per core. Replicate weight/adapters.
  - Base GEMM in fp8 e4m3 with DoubleRow perf mode (2 K-rows per PE
    cell pass): W pre-quantized on host at scale 64 (W*64 ~ N(0,1)
    fits e4m3), x shipped as bf16 in DoubleRow layout and cast to fp8
    on the DVE engine. PSUM accumulates 64*(x@W).
  - LoRA path needs better-than-fp8 x: the bf16 x feeds the adapter
    prologue a_allT = A_cat @ x^T (A stacked 8x16 rows). ams =
    a_allT * (smask*64) keeps the LoRA delta in the 64x domain, so
    one extra bf16 K-step per output tile lands it in the same PSUM
    accumulation group as the base matmuls.
  - Drain: DVE copies PSUM*(1/64) to SBUF, adds bf16 bias, DMA out.
    n=0 runs k-outer with the prologue fused; n>=1 runs m-outer so
    each tile's drain hides under the next tile's matmuls.
  - HWDGE costs ~625ns per DMA instruction -> batch DMAs (A in 4
    groups, W per n-slice with the first split in quarters only to
    unblock the PE sooner). Phase A (n=0 sweep) is DMA-feed-bound on
    the 8MB x stream, so its DMA issue order is critical-path-first
    (x chunk 0, A group 0, W quarter 0) with everything consumed at
    k2=15 (smask, bt/bias n=0 slices) staggered late and the bulk
    bt/bias + w[1] queued at the phase boundary. A 12-matmul warmup
    on zeroed SBUF absorbs the initial DMA latency + PE ramp.
  - Measured on hardware vs the fp32 reference: rel err 1.80e-2 <
    2e-2 gate. Deterministic: fp8 bits are produced host-side/on-chip
    with RNE; PE multiplies e4m3 exactly and accumulates fp32.
"""

import numpy as np
import ml_dtypes

import concourse.bass as bass
import concourse.bacc as bacc
import concourse.mybir as mybir
import concourse.tile as tile
from concourse.bass_utils import run_bass_kernel_spmd

SEQ, D_IN, D_OUT, RANK, N_ADAPTERS = 8192, 4096, 4096, 16, 8
N_CORES = 8
T = SEQ // N_CORES          # 1024 tokens per core
P = 128                     # partitions
FD = 512                    # psum tile free dim
KO = D_IN // P              # 32 single contraction tiles (prologue)
KO2 = D_IN // (2 * P)       # 16 DoubleRow contraction tiles (base)
NT = D_OUT // FD            # 8 output column chunks
MT = T // P                 # 8 token tiles per core
J = N_ADAPTERS * RANK       # 128 stacked adapter rows
AG = 4                      # A-tensor DMA groups
KOG = KO // AG              # ko's per A group (8)
WSCALE = 64.0               # fp8 scale for W (W*64 ~ N(0,1))
F32 = mybir.dt.float32
BF16 = mybir.dt.bfloat16
FP8 = mybir.dt.float8e4
DR = mybir.MatmulPerfMode.DoubleRow

_NC_CACHE = {}


def _build_nc():
    if "nc" in _NC_CACHE:
        return _NC_CACHE["nc"]
    nc = bacc.Bacc(None, target_bir_lowering=False, debug=False)
    xbf = nc.dram_tensor("xbf", [KO2, P, 2, T], BF16, kind="ExternalInput")
    w = nc.dram_tensor("w", [NT, P, KO2, 2, FD], FP8, kind="ExternalInput")
    biasb = nc.dram_tensor("biasb", [P, NT, FD], BF16, kind="ExternalInput")
    at = nc.dram_tensor("at", [AG, P, KOG, J], BF16, kind="ExternalInput")
    bt = nc.dram_tensor("bt", [J, NT, FD], BF16, kind="ExternalInput")
    smask = nc.dram_tensor("smask", [J, T], BF16, kind="ExternalInput")
    out = nc.dram_tensor("out", [T, D_OUT], BF16, kind="ExternalOutput")

    with tile.TileContext(nc) as tc:
        with (
            tc.tile_pool(name="xqp", bufs=1) as xq_pool,
            tc.tile_pool(name="xbp", bufs=8) as xbf_pool,
            tc.tile_pool(name="wp", bufs=3) as w_pool,
            tc.tile_pool(name="ap", bufs=2) as a_pool,
            tc.tile_pool(name="outp", bufs=8) as out_pool,
            tc.tile_pool(name="misc", bufs=1) as misc_pool,
            tc.tile_pool(name="psum", bufs=8, space="PSUM") as psum_pool,
        ):
            xbf_v = xbf[:]
            w_v = w[:]
            at_v = at[:]
            out_v = out[:]

            # tiles only; DMAs issued mid-phase-A (needed first at k2=15)
            smask_sb = misc_pool.tile([J, T], BF16, tag="smask")
            bt_sb = misc_pool.tile([J, NT, FD], BF16, tag="bt")
            bias_sb = misc_pool.tile([P, NT, FD], BF16, tag="bias")
            # a_allT * smask64 (bf16: matmul can't mix 32-bit and 16-bit ins)
            ams = misc_pool.tile([J, T], BF16, tag="ams")
            # resident fp8 x in DoubleRow layout [p, k2, j, t]
            xq = xq_pool.tile([P, KO2, 2, T], FP8, tag="xq")

            NCH = T // FD  # a_allT token chunks (2)
            psa = [None] * NCH

            # PE warmup: ~5us of throwaway matmuls during the initial DMA
            # wait so the p-state/HAM ramp is spent before real work
            warm_sb = misc_pool.tile([P, FD + P], BF16, tag="warm")
            nc.vector.memset(warm_sb[:], 0.0)
            ps_warm = psum_pool.tile([P, FD], F32, tag="ps", name="ps_warm")
            NWARM = 6  # memset chain (~1.6us) + 6 mms ends as xbf[0] lands
            for wi in range(NWARM):
                nc.tensor.matmul(
                    ps_warm[:], warm_sb[:, FD:], warm_sb[:, :FD],
                    start=(wi == 0), stop=(wi == NWARM - 1),
                )

            def drain_tile(ps, m, n, chunks=1):
                # bf16 output: halves the 16MB out-stream; +8e-4 rel err
                o_sb = out_pool.tile([P, FD], BF16, tag="o")
                cf = FD // chunks
                for ci in range(chunks):
                    sl = slice(ci * cf, (ci + 1) * cf)
                    # o = psum/64 + bias, single DVE op
                    nc.vector.scalar_tensor_tensor(
                        out=o_sb[:, sl], in0=ps[:, sl], scalar=1.0 / WSCALE,
                        in1=bias_sb[:, n, sl],
                        op0=mybir.AluOpType.mult, op1=mybir.AluOpType.add,
                    )
                    nc.sync.dma_start(
                        out_v[m * P:(m + 1) * P,
                              n * FD + ci * cf:n * FD + (ci + 1) * cf],
                        o_sb[:, sl],
                    )

            w1_sb = None  # fetched at the end of phase A
            for n in range(NT):
                if n == 1:
                    w_sb = w1_sb
                    nc.sync.dma_start(bt_sb[:, 1:], bt[:, 1:])
                    nc.sync.dma_start(bias_sb[:, 1:], biasb[:, 1:])
                else:
                    w_sb = w_pool.tile([P, KO2, 2, FD], FP8, tag="w",
                                       name=f"w_{n}")
                    if n != 0:
                        nc.sync.dma_start(w_sb[:], w_v[n])

                if n == 0:
                    # k-outer: prologue (A @ x^T, bf16) + x cast + base
                    # matmuls for m 0..5 share the k sweep; m 6,7 follow in
                    # a second sweep whose matmuls hide the m 0..5 drains.
                    # Base matmuls trail the prologue by LAG k2-steps so
                    # they never wait on a just-issued W quarter.
                    LAG = 2
                    for c in range(NCH):
                        psa[c] = psum_pool.tile([J, FD], F32, tag="ps",
                                                name=f"psa_{c}")
                    pss = {m: psum_pool.tile([P, FD], F32, tag="ps",
                                             name=f"ps_0_{m}")
                           for m in range(6)}
                    PF = 4  # xbf chunks prefetched ahead
                    xb_tiles = {}

                    def fetch_xb(k2):
                        xb_tiles[k2] = xbf_pool.tile(
                            [P, 2, T], BF16, tag="xb", name=f"xb_{k2}"
                        )
                        # j-halves land separately: prologue j=0 starts on
                        # half the chunk
                        nc.sync.dma_start(xb_tiles[k2][:, 0], xbf_v[k2, :, 0])
                        nc.sync.dma_start(xb_tiles[k2][:, 1], xbf_v[k2, :, 1])

                    def fetch_a(g):
                        t_ = a_pool.tile([P, KOG, J], BF16, tag="a",
                                         name=f"a_{g}")
                        nc.sync.dma_start(t_[:], at_v[g])
                        return t_

                    def fetch_wq(q):
                        nc.sync.dma_start(
                            w_sb[:, 4 * q:4 * (q + 1)],
                            w_v[n, :, 4 * q:4 * (q + 1)],
                        )

                    # critical-path-ordered start: x0, A0, x1, Wq0, x2, x3
                    fetch_xb(0)
                    a_tiles = {0: fetch_a(0)}
                    fetch_xb(1)
                    fetch_wq(0)
                    fetch_xb(2)
                    fetch_xb(3)
                    for k2 in range(KO2):
                        last_k = k2 == KO2 - 1
                        if k2 + PF < KO2:
                            fetch_xb(k2 + PF)
                        xb_sb = xb_tiles.pop(k2)
                        # cast bf16 -> fp8 on DVE (1.07us < 1.49us PE pace),
                        # per j-half so each starts as its DMA lands
                        nc.vector.tensor_copy(xq[:, k2, 0], xb_sb[:, 0])
                        nc.vector.tensor_copy(xq[:, k2, 1], xb_sb[:, 1])
                        for j in range(2):
                            ko = 2 * k2 + j
                            a_sb = a_tiles[ko // KOG]
                            for c in range(NCH):
                                nc.tensor.matmul(
                                    psa[c][:], a_sb[:, ko % KOG],
                                    xb_sb[:, j, c * FD:(c + 1) * FD],
                                    start=(ko == 0), stop=(ko == KO - 1),
                                )
                        # stagger non-critical DMAs behind the x stream:
                        # A groups and w quarters ahead of their consumers,
                        # the k2=15-consumed tensors (smask/bt/bias) late
                        if k2 in (2, 6, 10):
                            fetch_wq(k2 // 4 + 1)
                        if k2 in (3, 7, 11):
                            g = k2 // 4 + 1
                            a_tiles[g] = fetch_a(g)
                        elif k2 == 9:
                            nc.sync.dma_start(smask_sb[:], smask[:])
                        elif k2 == 12:
                            # phase A only needs the n=0 slices; the rest
                            # streams once A-critical bytes are all queued
                            nc.sync.dma_start(bt_sb[:, 0], bt[:, 0])
                        elif k2 == 13:
                            nc.sync.dma_start(bias_sb[:, 0], biasb[:, 0])
                        if last_k:
                            for c in range(NCH):
                                nc.vector.tensor_mul(
                                    out=ams[:, c * FD:(c + 1) * FD],
                                    in0=psa[c][:],
                                    in1=smask_sb[:, c * FD:(c + 1) * FD],
                                )
                        if k2 >= LAG:
                            k2b = k2 - LAG
                            for m in range(6):
                                nc.tensor.matmul(
                                    pss[m][:], xq[:, k2b, :, m * P:(m + 1) * P],
                                    w_sb[:, k2b], start=(k2b == 0), stop=False,
                                    perf_mode=DR,
                                )
                    for k2b in range(KO2 - LAG, KO2):
                        for m in range(6):
                            nc.tensor.matmul(
                                pss[m][:], xq[:, k2b, :, m * P:(m + 1) * P],
                                w_sb[:, k2b], start=(k2b == 0), stop=False,
                                perf_mode=DR,
                            )
                    # w[1] next in the DMA queue, ahead of the drain
                    # out-DMAs: lands during the g2 sweep, so n=1 never waits
                    w1_sb = w_pool.tile([P, KO2, 2, FD], FP8, tag="w",
                                        name="w_1")
                    nc.sync.dma_start(w1_sb[:], w_v[1])
                    for m in range(6):
                        nc.tensor.matmul(
                            pss[m][:], ams[:, m * P:(m + 1) * P],
                            bt_sb[:, n], start=False, stop=True,
                        )
                        drain_tile(pss[m], m, n)
                    for m in range(6, MT):
                        ps = psum_pool.tile([P, FD], F32, tag="ps",
                                            name=f"ps_0_{m}")
                        for k2 in range(KO2):
                            nc.tensor.matmul(
                                ps[:], xq[:, k2, :, m * P:(m + 1) * P],
                                w_sb[:, k2], start=(k2 == 0), stop=False,
                                perf_mode=DR,
                            )
                        nc.tensor.matmul(
                            ps[:], ams[:, m * P:(m + 1) * P], bt_sb[:, n],
                            start=False, stop=True,
                        )
                        drain_tile(ps, m, n)
                else:
                    # m-outer: each tile closes right after its k sweep, so
                    # DVE drains + out DMA hide under the next tile's matmuls
                    for m in range(MT):
                        ps = psum_pool.tile([P, FD], F32, tag="ps",
                                            name=f"ps_{n}_{m}")
                        for k2 in range(KO2):
                            nc.tensor.matmul(
                                ps[:], xq[:, k2, :, m * P:(m + 1) * P],
                                w_sb[:, k2], start=(k2 == 0), stop=False,
                                perf_mode=DR,
                            )
                        nc.tensor.matmul(
                            ps[:], ams[:, m * P:(m + 1) * P], bt_sb[:, n],
                            start=False, stop=True,
                        )
                        drain_tile(ps, m, n)

    nc.compile()
    _NC_CACHE["nc"] = nc
    return nc


def _prep_in_maps(x, weight, bias, A_buffer, B_buffer, scalings, token_indices):
    x = np.asarray(x, np.float32)
    weight = np.asarray(weight, np.float32)
    bias = np.asarray(bias, np.float32)
    A_buffer = np.asarray(A_buffer, np.float32)
    B_buffer = np.asarray(B_buffer, np.float32)
    scalings = np.asarray(scalings, np.float32)
    token_indices = np.asarray(token_indices)

    # x^T in DoubleRow layout [k2, p, j, t], bf16
    xT = x.T.reshape(KO2, 2, P, SEQ).transpose(0, 2, 1, 3)
    xbf_full = np.ascontiguousarray(xT.astype(ml_dtypes.bfloat16))
    # W*64 quantized to e4m3, DoubleRow layout [n, p, k2, j, f]
    wq = np.clip(weight * WSCALE, -240, 240).astype(ml_dtypes.float8_e4m3)
    w_t = np.ascontiguousarray(
        wq.reshape(KO2, 2, P, NT, FD).transpose(3, 2, 0, 1, 4)
    )
    biasb = np.ascontiguousarray(
        np.broadcast_to(
            bias.reshape(1, NT, FD), (P, NT, FD)
        ).astype(ml_dtypes.bfloat16)
    )
    A_cat = A_buffer.reshape(J, D_IN)
    # [AG, P, KOG, J]: per-group contiguous per partition
    at = np.ascontiguousarray(
        A_cat.T.reshape(AG, KOG, P, J).transpose(0, 2, 1, 3)
        .astype(ml_dtypes.bfloat16)
    )
    bt = np.ascontiguousarray(
        B_buffer.transpose(0, 2, 1).reshape(J, NT, FD).astype(ml_dtypes.bfloat16)
    )
    adapter_of_row = (np.arange(J) // RANK).astype(token_indices.dtype)
    smask_full = (
        (token_indices[None, :] == adapter_of_row[:, None]).astype(np.float32)
        * (scalings[None, :] * np.float32(WSCALE))
    ).astype(ml_dtypes.bfloat16)  # [J, SEQ], includes the 64x domain scale

    in_maps = []
    for c in range(N_CORES):
        sl = slice(c * T, (c + 1) * T)
        in_maps.append({
            "xbf": np.ascontiguousarray(xbf_full[:, :, :, sl]),
            "w": w_t,
            "biasb": biasb,
            "at": at,
            "bt": bt,
            "smask": np.ascontiguousarray(smask_full[:, sl]),
        })
    return in_maps


def _run(inputs, trace=False):
    nc = _build_nc()
    in_maps = _prep_in_maps(**inputs)
    res = run_bass_kernel_spmd(
        nc, in_maps, core_ids=list(range(N_CORES)), trace=trace
    )
    out = np.concatenate([r["out"] for r in res.results], axis=0)
    return out.astype(np.float32), res


def kernel(**inputs) -> np.ndarray:
    out, _ = _run(inputs, trace=False)
    return out
